# revision 59
# baseline (speedup 1.0000x reference)
"""Causal self-attention (B=2, T=2048, C=1024, NH=16, HD=64) on 8 trn2 NeuronCores.

Sharding: 2 batch groups x 4 head-groups. Core g (0..7) handles batch b=g//4
and heads [4*(g%4), 4*(g%4)+4). Each core computes its 4 heads' attention and a
partial projection (row-split W_proj); the host sums 4 partials per batch.

v2: software-pipelined emission so the PE never queues behind EXP latency:
  - x DMA'd in t-blocks; QKV for t-block tb interleaves with attention i-block
    tb-1, so ACT's exp pipeline starts ~11us into the kernel.
  - scores(jc+1) emitted before AV/dn(jc): EXP latency hides under the next
    chunk's scores matmuls; PE stays dense (HAM stays warm at 2.4GHz).
  - pt split per head-pair so AV pair p waits only its own exp.
  - reciprocal_approx_fast, rbc evacuation on ACT, bf16 output (half DMA),
    projection for i-block ib interleaved into i-block ib+1's jc loop.
"""

import numpy as np

B, T, C, NH, HD = 2, 2048, 1024, 16, 64
NCORES = 8
HPC = 4          # heads per core
IBS = 512        # i-block (query block) size
NIB = T // IBS   # 4 i-blocks
JCS = 128        # j-chunk (key chunk) size

_cache = {}


def _patch_tile_drain():
    """This walrus build can't encode multi-wait InstDrain: split the Tile
    tail drain into a chain of single-wait drains."""
    import concourse.tile as _tile
    if getattr(_tile.TileContext, "_drain_patched", False):
        return
    import bass_rust as _br
    from concourse.vector_clock import ScopedClock

    def _drain_and_barrier(self, tick_clock, wait_clock):
        nc = self.nc
        drain_inst = nc.sync.drain()
        wait_clock.add_sem_waits(
            drain_inst.ins, ScopedClock({None: tick_clock.global_clock})
        )
        si = drain_inst.ins.sync_info
        waits = list(si.on_wait or [])
        if len(waits) > 1:
            si.on_wait = waits[:1]
            for w in waits[1:]:
                extra = nc.sync.drain()
                extra.ins.sync_info = _br.SyncInfo(on_wait=[w], on_update=[])
        nc.all_engine_barrier()
        assert self.sems is not None
        popped = nc._tile_sem_poison_stack.pop()
        assert popped is self._sem_poison
        nc.clear_and_free_semaphores(list(self.sems.allocated().values()))
        nc.all_engine_barrier()

    _tile.TileContext._drain_and_barrier = _drain_and_barrier

    # This walrus also refuses >1 sem wait on ANY instruction: peel extra
    # waits onto ENGINE_NOP carriers inserted just before, same engine/bb.
    _orig_add = _tile.TileContext._add_instruction

    def _add_instruction(self, inst):
        si = getattr(inst, "sync_info", None)
        if si is not None and si.on_wait and len(si.on_wait) > 1:
            waits = list(si.on_wait)
            si.on_wait = waits[-1:]
            import concourse.mybir as _mb
            for w in waits[:-1]:
                nop = _mb.InstEventSemaphore(
                    name=self.nc.get_next_instruction_name(), ins=[], outs=[])
                nop.engine = inst.engine
                nop.sync_info = _br.SyncInfo(on_wait=[w], on_update=[])
                _orig_add(self, nop)
        _orig_add(self, inst)

    _tile.TileContext._add_instruction = _add_instruction
    _tile.TileContext._drain_patched = True


def build_nc():
    import concourse.bass as bass
    import concourse.mybir as mybir
    import concourse.tile as tile
    from contextlib import ExitStack

    _patch_tile_drain()
    dt = mybir.dt
    f32, bf16 = dt.float32, dt.bfloat16
    AL = mybir.AluOpType
    Exp = mybir.ActivationFunctionType.Exp
    Ln = mybir.ActivationFunctionType.Ln
    nc = bass.Bass()

    xt = nc.declare_dram_parameter("xt", [C, T], bf16, isOutput=False)
    wd = {}
    for name, w in (("qe", 128), ("qo", 128), ("ke", 128), ("ko", 128),
                    ("v", 256)):
        wd[name] = nc.declare_dram_parameter(f"w{name}", [C, w], bf16,
                                             isOutput=False)
    cosd = nc.declare_dram_parameter("cosd", [128, T], bf16, isOutput=False)
    sind = nc.declare_dram_parameter("sind", [128, T], bf16, isOutput=False)
    maskA = nc.declare_dram_parameter("maskA", [128, 128], bf16, isOutput=False)
    wp = nc.declare_dram_parameter("wp", [256, C], bf16, isOutput=False)
    out = nc.declare_dram_parameter("out", [T, C], bf16, isOutput=True)

    scale = 1.0 / float(np.sqrt(HD))

    with tile.TileContext(nc) as tc, ExitStack() as ctx:
        const = ctx.enter_context(tc.tile_pool(name="const", bufs=1))

        # ---------- DMA: two HWDGE rings (sync + scalar), big transfers,
        # earliest-needed first ----------
        w_sb = {}
        for name in ("ke", "ko", "qe", "qo"):
            w_sb[name] = const.tile([128, 8, 128], bf16, tag=f"w_{name}",
                                    name=f"w_{name}")
        cos_sb = const.tile([128, T], bf16)
        sin_sb = const.tile([128, T], bf16)
        xt_sb = const.tile([128, 8, T], bf16)
        xt_r = xt[:].rearrange("(cc p) t -> p cc t", p=128)
        m0_sb = const.tile([128, 128], bf16)
        wvt = const.tile([128, 8, 256], bf16, tag="w_v", name="w_v")
        w_sb["v"] = wvt
        wp_sb = const.tile([128, 2, C], bf16)
        tsl0 = slice(0, 512)
        # sync ring: k weights, then x t-blocks
        for name in ("ke", "ko"):
            nc.sync.dma_start(w_sb[name][:], wd[name][:].rearrange(
                "(cc p) j -> p cc j", p=128))
        nc.sync.dma_start(xt_sb[:, 0:4, tsl0], xt_r[:, 0:4, tsl0])
        nc.sync.dma_start(xt_sb[:, 4:8, tsl0], xt_r[:, 4:8, tsl0])
        for tb in range(1, 4):
            tsl = slice(tb * 512, (tb + 1) * 512)
            nc.sync.dma_start(xt_sb[:, :, tsl], xt_r[:, :, tsl])
        # scalar ring: q weights, rope tables, mask, v/proj weights
        for name in ("qe", "qo"):
            nc.scalar.dma_start(w_sb[name][:], wd[name][:].rearrange(
                "(cc p) j -> p cc j", p=128))
        nc.scalar.dma_start(cos_sb[:, tsl0], cosd[:, tsl0])
        nc.scalar.dma_start(sin_sb[:, tsl0], sind[:, tsl0])
        nc.scalar.dma_start(m0_sb[:], maskA[:])
        nc.scalar.dma_start(wvt[:], wd["v"][:].rearrange(
            "(cc p) j -> p cc j", p=128))
        for tb in range(1, 4):
            tsl = slice(tb * 512, (tb + 1) * 512)
            nc.scalar.dma_start(cos_sb[:, tsl], cosd[:, tsl])
            nc.scalar.dma_start(sin_sb[:, tsl], sind[:, tsl])
        nc.scalar.dma_start(wp_sb[:], wp[:].rearrange(
            "(cc p) co -> p cc co", p=128))

        ones_sb = const.tile([128, 64], bf16)
        nc.gpsimd.memset(ones_sb[:], 1.0)
        # warm the ACT exp table-set during input DMA
        warm_sb = const.tile([128, 16], f32)
        nc.vector.memset(warm_sb[:], 0.0)
        nc.scalar.activation(warm_sb[:], warm_sb[:], Exp)

        # rope'd Q^T/K^T halves; rows = 4 heads x 32 dims
        qke = {n: const.tile([128, T], bf16, tag=f"r_{n}", name=f"r_{n}")
               for n in ("qe", "qo", "ke", "ko")}
        v_sb = const.tile([128, 16, 256], bf16)   # V natural, t-chunks
        yab = const.tile([128, T], bf16)          # y^T heads A,B (normalized)
        ycd = const.tile([128, T], bf16)          # y^T heads C,D

        # pools (PSUM: sc 4 banks + y 2 + dn/rb 1 + blk 1 = 8)
        sc_ps = ctx.enter_context(tc.tile_pool(name="sc_ps", bufs=2, space="PSUM"))
        y_psp = ctx.enter_context(tc.tile_pool(name="y_ps", bufs=1, space="PSUM"))
        dn_ps = ctx.enter_context(tc.tile_pool(name="dn_ps", bufs=1, space="PSUM"))
        blk_ps = ctx.enter_context(tc.tile_pool(name="blk_ps", bufs=1, space="PSUM"))
        p_sbp = ctx.enter_context(tc.tile_pool(name="p_sb", bufs=4))
        rope_t = ctx.enter_context(tc.tile_pool(name="rope_t", bufs=3))
        r_sbp = ctx.enter_context(tc.tile_pool(name="r_sb", bufs=2))
        rbc_sbp = ctx.enter_context(tc.tile_pool(name="rbc_sb", bufs=3))
        pj_sbp = ctx.enter_context(tc.tile_pool(name="pj_sb", bufs=4))

        # ---------- emission helpers ----------
        def emit_qk(mk, tb):
            """QKV matmuls + rope for (q|k, t-block tb)."""
            tsl = slice(tb * 512, (tb + 1) * 512)
            ps = sc_ps.tile([128, 2, 512], f32, tag="scps")
            for cc in range(8):
                nc.tensor.matmul(ps[:, 0, :], lhsT=w_sb[mk + "e"][:, cc, :],
                                 rhs=xt_sb[:, cc, tsl],
                                 start=(cc == 0), stop=(cc == 7))
            for cc in range(8):
                nc.tensor.matmul(ps[:, 1, :], lhsT=w_sb[mk + "o"][:, cc, :],
                                 rhs=xt_sb[:, cc, tsl],
                                 start=(cc == 0), stop=(cc == 7))
            # evacuate PSUM via ACT (idle during QKV), then rope on SBUF
            # where DVE fp32 runs full rate.  E'=E*cos-O*sin ; O'=O*cos+E*sin
            cpe = rope_t.tile([128, 512], f32, tag="cpe")
            nc.scalar.copy(cpe[:], ps[:, 0, :])
            cpo = rope_t.tile([128, 512], f32, tag="cpo")
            nc.vector.tensor_copy(cpo[:], ps[:, 1, :])
            a = rope_t.tile([128, 512], f32, tag="ra")
            nc.vector.tensor_tensor(a[:], cpe[:], cos_sb[:, tsl], AL.mult)
            b = rope_t.tile([128, 512], f32, tag="rb")
            nc.vector.tensor_tensor(b[:], cpo[:], sin_sb[:, tsl], AL.mult)
            nc.vector.tensor_tensor(qke[mk + "e"][:, tsl], a[:], b[:],
                                    AL.subtract)
            c = rope_t.tile([128, 512], f32, tag="rc")
            nc.vector.tensor_tensor(c[:], cpo[:], cos_sb[:, tsl], AL.mult)
            d = rope_t.tile([128, 512], f32, tag="rd")
            nc.vector.tensor_tensor(d[:], cpe[:], sin_sb[:, tsl], AL.mult)
            nc.vector.tensor_tensor(qke[mk + "o"][:, tsl], c[:], d[:], AL.add)

        def emit_vpair(vp):
            """V matmuls for t-chunks 2*vp, 2*vp+1 through one blk rotation."""
            bt = blk_ps.tile([128, 512], f32, tag="blk")
            for k in range(2):
                tcx = 2 * vp + k
                tchunk = slice(tcx * 128, (tcx + 1) * 128)
                for cc in range(8):
                    nc.tensor.matmul(bt[:, 256 * k:256 * (k + 1)],
                                     lhsT=xt_sb[:, cc, tchunk],
                                     rhs=w_sb["v"][:, cc, :],
                                     start=(cc == 0), stop=(cc == 7),
                                     skip_group_check=True)
                nc.vector.tensor_copy(v_sb[:, tcx, :],
                                      bt[:, 256 * k:256 * (k + 1)])

        qe_sb, qo_sb = qke["qe"], qke["qo"]
        ke_sb, ko_sb = qke["ke"], qke["ko"]

        def emit_scores(ib, jc):
            """scores + exp for (ib, jc); returns the two pt pair-tiles."""
            njc = 4 * ib + 4
            jsl = slice(jc * JCS, (jc + 1) * JCS)
            jcd = jc - 4 * ib
            off = 128 * jcd if jcd > 0 else 0
            osl = slice(off, 512)
            iosl = slice(ib * IBS + off, (ib + 1) * IBS)
            pts = []
            for half in range(2):
                sch = sc_ps.tile([128, 2, 512], f32, tag="scps")
                for hh in range(2):
                    h = 2 * half + hh
                    hsl = slice(32 * h, 32 * h + 32)
                    nc.tensor.matmul(
                        sch[:, hh, osl], lhsT=ke_sb[hsl, jsl],
                        rhs=qe_sb[hsl, iosl], start=True, stop=False,
                        tile_position=(32 * h, 0))
                for hh in range(2):
                    h = 2 * half + hh
                    hsl = slice(32 * h, 32 * h + 32)
                    nc.tensor.matmul(
                        sch[:, hh, osl], lhsT=ko_sb[hsl, jsl],
                        rhs=qo_sb[hsl, iosl], start=False, stop=True,
                        tile_position=(32 * h, 0))
                pt = p_sbp.tile([128, 2, 512], bf16, tag=f"pt{half}")
                nc.scalar.activation(pt[:, :, osl], sch[:, :, osl], Exp,
                                     scale=scale)
                if jcd >= 0:
                    tsl = slice(off, off + 128)
                    nc.vector.tensor_tensor(
                        pt[:, :, tsl], pt[:, :, tsl],
                        m0_sb[:, None, :].broadcast_to([128, 2, 128]),
                        AL.mult)
                pts.append(pt)
            return pts

        def emit_avdn(ib, jc, pts, y_ps, dn):
            njc = 4 * ib + 4
            jcd = jc - 4 * ib
            off = 128 * jcd if jcd > 0 else 0
            osl = slice(off, 512)
            for pair in range(2):
                for k in range(2):
                    h = 2 * pair + k
                    nc.tensor.matmul(
                        y_ps[64 * k:64 * (k + 1), pair, osl],
                        lhsT=v_sb[:, jc, 64 * h:64 * (h + 1)],
                        rhs=pts[pair][:, k, osl],
                        start=(jc == 0), stop=(jc == njc - 1),
                        tile_position=(0, 64 * k),
                        skip_group_check=True)
            for h in range(4):
                nc.tensor.matmul(
                    dn[32 * h:32 * h + 1, osl],
                    lhsT=ones_sb[:, 0:1],
                    rhs=pts[h // 2][:, h % 2, osl],
                    start=(jc == 0), stop=(jc == njc - 1),
                    tile_position=(0, 32 * h),
                    skip_group_check=True)

        def emit_tail1(ib, dn):
            """1/dn as exp(-ln(dn)), both on ACT: Ln reads PSUM directly
            (fast bank release), Exp writes bf16 -- ~1.5us chain with no
            DVE involvement, vs copy+iterative-reciprocal (~4.4us)."""
            t = r_sbp.tile([128, 512], f32, tag="dnsb")
            nc.scalar.activation(t[:], dn[:], Ln)
            r16 = r_sbp.tile([128, 512], bf16, tag="r16")
            nc.scalar.activation(r16[:], t[:], Exp, scale=-1.0)
            return r16

        def emit_tail2(ib, y_ps, r16):
            """rbcast + normalize; deferred into the next ib's fill stream
            so the PE never queues behind the reciprocal."""
            isl = slice(ib * IBS, (ib + 1) * IBS)
            for pair, ytile in ((0, yab), (1, ycd)):
                rb = blk_ps.tile([128, 512], f32, tag="blk")
                for k in range(2):
                    h = 2 * pair + k
                    nc.tensor.matmul(
                        rb[64 * k:64 * (k + 1), :],
                        lhsT=ones_sb[32 * h:32 * h + 1, :],
                        rhs=r16[32 * h:32 * h + 1, :],
                        start=True, stop=True,
                        tile_position=(32 * h, 64 * k),
                        skip_group_check=True)
                rbc = rbc_sbp.tile([128, 512], f32, tag="rbc")
                nc.vector.tensor_copy(rbc[:], rb[:])
                nc.vector.tensor_tensor(ytile[:, isl], y_ps[:, pair, :],
                                        rbc[:], AL.mult)

        def emit_proj_block(tcx, cob, fast=False):
            tchunk = slice(tcx * 128, (tcx + 1) * 128)
            cosl = slice(cob * 512, (cob + 1) * 512)
            if fast:  # epilogue: sc pool is free, use it double-buffered
                ppt = sc_ps.tile([128, 2, 512], f32, tag="scps", name="pjf")
                pp = ppt[:, 0, :]
            else:
                pp = blk_ps.tile([128, 512], f32, tag="blk")
            nc.tensor.matmul(pp[:], lhsT=yab[:, tchunk],
                             rhs=wp_sb[:, 0, cosl], start=True, stop=False)
            nc.tensor.matmul(pp[:], lhsT=ycd[:, tchunk],
                             rhs=wp_sb[:, 1, cosl], start=False, stop=True)
            ps = pj_sbp.tile([128, 512], bf16, tag="pjsb")
            nc.vector.tensor_copy(ps[:], pp[:])
            nc.sync.dma_start(out[tchunk, cosl], ps[:])

        # ---------- main emission schedule ----------
        # Fill queue: QKV/V/proj work drained one unit per jc iteration,
        # emitted between scores(jc) and avdn(jc-1) so it lands exactly in
        # the PE's exp-wait window instead of queuing behind it.
        fillq = []
        tail2_pending = []   # at most one deferred (ib, y_ps, r16)

        def flush_tail2():
            if tail2_pending:
                emit_tail2(*tail2_pending.pop())

        def emit_attention(ib):
            njc = 4 * ib + 4
            acc = {}

            def ensure_acc():
                if "y" not in acc:
                    # previous ib's rbcast/normalize must be emitted before
                    # this ib claims the single y_ps buffer
                    flush_tail2()
                    acc["y"] = y_psp.tile([128, 2, 512], f32, tag="yps",
                                          name="yps")
                    acc["dn"] = dn_ps.tile([128, 512], f32, tag="dnrb",
                                           name="dnrb")

            pending = None
            for jc in range(njc):
                pts = emit_scores(ib, jc)
                if fillq:
                    fillq.pop(0)()
                if pending is not None:
                    ensure_acc()
                    emit_avdn(ib, pending[0], pending[1], acc["y"], acc["dn"])
                pending = (jc, pts)
            ensure_acc()
            emit_avdn(ib, pending[0], pending[1], acc["y"], acc["dn"])
            r16 = emit_tail1(ib, acc["dn"])
            tail2_pending.append((ib, acc["y"], r16))
            for tcx in range(4 * ib, 4 * ib + 4):
                for cob in range(2):
                    fillq.append(
                        lambda fast=False, t=tcx, c=cob:
                        emit_proj_block(t, c, fast))

        # eager prologue: QKV for t-block 0 only; all the rest of QKV/V
        # rides the fill queue so attention(0)'s exp pipeline starts ASAP
        for mk in ("k", "q"):
            emit_qk(mk, 0)
        fillq.extend([
            lambda: emit_vpair(0), lambda: emit_vpair(1),
            lambda: emit_qk("k", 1), lambda: emit_qk("q", 1),
        ])
        emit_attention(0)
        fillq[:0] = [
            lambda: emit_qk("k", 2), lambda: emit_qk("q", 2),
            lambda: emit_vpair(2), lambda: emit_vpair(3),
        ]
        emit_attention(1)
        fillq[:0] = [
            lambda: emit_qk("k", 3), lambda: emit_qk("q", 3),
            lambda: emit_vpair(4), lambda: emit_vpair(5),
        ]
        emit_attention(2)
        fillq[:0] = [
            lambda: emit_vpair(6), lambda: emit_vpair(7),
        ]
        emit_attention(3)
        flush_tail2()
        while fillq:  # epilogue: proj(ib3) blocks, pipelined through sc pool
            fillq.pop(0)(fast=True)
    return nc


def _host_prep(x, cos, sin, W_attn, W_proj):
    """Build the 8 per-core input maps (pure data movement / layout prep)."""
    import ml_dtypes
    bf16 = ml_dtypes.bfloat16
    x = np.asarray(x)
    cos = np.asarray(cos)
    sin = np.asarray(sin)
    W_attn = np.asarray(W_attn)
    W_proj = np.asarray(W_proj)

    cosf = np.ascontiguousarray(cos[0, 0][:, 0::2].T.astype(np.float32))  # [32,T]
    sinf = np.ascontiguousarray(sin[0, 0][:, 0::2].T.astype(np.float32))
    cosd = np.tile(cosf, (4, 1)).astype(bf16)  # [128, T]
    sind = np.tile(sinf, (4, 1)).astype(bf16)

    mA = (np.arange(128)[:, None] <= np.arange(128)[None, :]).astype(np.float32)

    ev = np.arange(0, HD, 2)
    od = np.arange(1, HD, 2)
    Wq, Wk, Wv = W_attn[:, 0:C], W_attn[:, C:2 * C], W_attn[:, 2 * C:3 * C]
    xt = [np.ascontiguousarray(x[b].T).astype(bf16) for b in range(B)]

    in_maps = []
    for g in range(NCORES):
        b, hg = g // HPC, g % HPC
        heads = [HPC * hg + i for i in range(HPC)]
        mk = lambda W, idx: np.ascontiguousarray(
            np.concatenate([W[:, 64 * h + idx] for h in heads], 1)).astype(bf16)
        in_maps.append({
            "xt": xt[b],
            "wqe": mk(Wq, ev), "wqo": mk(Wq, od),
            "wke": mk(Wk, ev), "wko": mk(Wk, od),
            "wv": mk(Wv, np.arange(HD)),
            "cosd": cosd, "sind": sind, "maskA": mA.astype(bf16),
            "wp": np.ascontiguousarray(np.concatenate(
                [W_proj[64 * h:64 * h + 64, :] for h in heads], 0)
            ).astype(bf16),
        })
    return in_maps


def _run(inputs, trace=False):
    from concourse.bass_utils import run_bass_kernel_spmd

    if "nc" not in _cache:
        _cache["nc"] = build_nc()
    nc = _cache["nc"]
    in_maps = _host_prep(**inputs)
    res = run_bass_kernel_spmd(
        nc, in_maps, core_ids=list(range(NCORES)), trace=trace)
    outp = np.stack([np.asarray(res.results[g]["out"], dtype=np.float32)
                     for g in range(NCORES)])
    full = np.stack([outp[4 * b:4 * b + 4].sum(axis=0) for b in range(B)])
    return full, res


def kernel(**inputs):
    full, _ = _run(inputs, trace=False)
    return full


# revision 61
# speedup vs baseline: 1.0107x; 1.0107x over previous
"""Causal self-attention (B=2, T=2048, C=1024, NH=16, HD=64) on 8 trn2 NeuronCores.

Sharding: 2 batch groups x 4 head-groups. Core g (0..7) handles batch b=g//4
and heads [4*(g%4), 4*(g%4)+4). Each core computes its 4 heads' attention and a
partial projection (row-split W_proj); the host sums 4 partials per batch.

v2: software-pipelined emission so the PE never queues behind EXP latency:
  - x DMA'd in t-blocks; QKV for t-block tb interleaves with attention i-block
    tb-1, so ACT's exp pipeline starts ~11us into the kernel.
  - scores(jc+1) emitted before AV/dn(jc): EXP latency hides under the next
    chunk's scores matmuls; PE stays dense (HAM stays warm at 2.4GHz).
  - pt split per head-pair so AV pair p waits only its own exp.
  - reciprocal_approx_fast, rbc evacuation on ACT, bf16 output (half DMA),
    projection for i-block ib interleaved into i-block ib+1's jc loop.
"""

import numpy as np

B, T, C, NH, HD = 2, 2048, 1024, 16, 64
NCORES = 8
HPC = 4          # heads per core
IBS = 512        # i-block (query block) size
NIB = T // IBS   # 4 i-blocks
JCS = 128        # j-chunk (key chunk) size

_cache = {}


def _patch_tile_drain():
    """This walrus build can't encode multi-wait InstDrain: split the Tile
    tail drain into a chain of single-wait drains."""
    import concourse.tile as _tile
    if getattr(_tile.TileContext, "_drain_patched", False):
        return
    import bass_rust as _br
    from concourse.vector_clock import ScopedClock

    def _drain_and_barrier(self, tick_clock, wait_clock):
        nc = self.nc
        drain_inst = nc.sync.drain()
        wait_clock.add_sem_waits(
            drain_inst.ins, ScopedClock({None: tick_clock.global_clock})
        )
        si = drain_inst.ins.sync_info
        waits = list(si.on_wait or [])
        if len(waits) > 1:
            si.on_wait = waits[:1]
            for w in waits[1:]:
                extra = nc.sync.drain()
                extra.ins.sync_info = _br.SyncInfo(on_wait=[w], on_update=[])
        nc.all_engine_barrier()
        assert self.sems is not None
        popped = nc._tile_sem_poison_stack.pop()
        assert popped is self._sem_poison
        nc.clear_and_free_semaphores(list(self.sems.allocated().values()))
        nc.all_engine_barrier()

    _tile.TileContext._drain_and_barrier = _drain_and_barrier

    # This walrus also refuses >1 sem wait on ANY instruction: peel extra
    # waits onto ENGINE_NOP carriers inserted just before, same engine/bb.
    _orig_add = _tile.TileContext._add_instruction

    def _add_instruction(self, inst):
        si = getattr(inst, "sync_info", None)
        if si is not None and si.on_wait and len(si.on_wait) > 1:
            waits = list(si.on_wait)
            si.on_wait = waits[-1:]
            import concourse.mybir as _mb
            for w in waits[:-1]:
                nop = _mb.InstEventSemaphore(
                    name=self.nc.get_next_instruction_name(), ins=[], outs=[])
                nop.engine = inst.engine
                nop.sync_info = _br.SyncInfo(on_wait=[w], on_update=[])
                _orig_add(self, nop)
        _orig_add(self, inst)

    _tile.TileContext._add_instruction = _add_instruction
    _tile.TileContext._drain_patched = True


def build_nc():
    import concourse.bass as bass
    import concourse.mybir as mybir
    import concourse.tile as tile
    from contextlib import ExitStack

    _patch_tile_drain()
    dt = mybir.dt
    f32, bf16 = dt.float32, dt.bfloat16
    AL = mybir.AluOpType
    Exp = mybir.ActivationFunctionType.Exp
    Ln = mybir.ActivationFunctionType.Ln
    nc = bass.Bass()

    xt = nc.declare_dram_parameter("xt", [C, T], bf16, isOutput=False)
    wd = {}
    for name, w in (("qe", 128), ("qo", 128), ("ke", 128), ("ko", 128),
                    ("v", 256)):
        wd[name] = nc.declare_dram_parameter(f"w{name}", [C, w], bf16,
                                             isOutput=False)
    cosd = nc.declare_dram_parameter("cosd", [128, T], bf16, isOutput=False)
    sind = nc.declare_dram_parameter("sind", [128, T], bf16, isOutput=False)
    maskA = nc.declare_dram_parameter("maskA", [128, 128], bf16, isOutput=False)
    wp = nc.declare_dram_parameter("wp", [256, C], bf16, isOutput=False)
    out = nc.declare_dram_parameter("out", [T, C], bf16, isOutput=True)

    scale = 1.0 / float(np.sqrt(HD))

    with tile.TileContext(nc) as tc, ExitStack() as ctx:
        const = ctx.enter_context(tc.tile_pool(name="const", bufs=1))

        # ---------- DMA: two HWDGE rings (sync + scalar), big transfers,
        # earliest-needed first ----------
        w_sb = {}
        for name in ("ke", "ko", "qe", "qo"):
            w_sb[name] = const.tile([128, 8, 128], bf16, tag=f"w_{name}",
                                    name=f"w_{name}")
        cos_sb = const.tile([128, T], bf16)
        sin_sb = const.tile([128, T], bf16)
        xt_sb = const.tile([128, 8, T], bf16)
        xt_r = xt[:].rearrange("(cc p) t -> p cc t", p=128)
        m0_sb = const.tile([128, 128], bf16)
        wvt = const.tile([128, 8, 256], bf16, tag="w_v", name="w_v")
        w_sb["v"] = wvt
        wp_sb = const.tile([128, 2, C], bf16)
        tsl0 = slice(0, 512)
        # sync ring: wke, then x tb0 first half (unblocks the first e-MMs),
        # then wko, then the rest
        nc.sync.dma_start(w_sb["ke"][:], wd["ke"][:].rearrange(
            "(cc p) j -> p cc j", p=128))
        nc.sync.dma_start(xt_sb[:, 0:4, tsl0], xt_r[:, 0:4, tsl0])
        nc.sync.dma_start(w_sb["ko"][:], wd["ko"][:].rearrange(
            "(cc p) j -> p cc j", p=128))
        nc.sync.dma_start(xt_sb[:, 4:8, tsl0], xt_r[:, 4:8, tsl0])
        for tb in range(1, 4):
            tsl = slice(tb * 512, (tb + 1) * 512)
            nc.sync.dma_start(xt_sb[:, :, tsl], xt_r[:, :, tsl])
        # scalar ring: q weights, rope tables, mask, v/proj weights
        for name in ("qe", "qo"):
            nc.scalar.dma_start(w_sb[name][:], wd[name][:].rearrange(
                "(cc p) j -> p cc j", p=128))
        nc.scalar.dma_start(cos_sb[:, tsl0], cosd[:, tsl0])
        nc.scalar.dma_start(sin_sb[:, tsl0], sind[:, tsl0])
        nc.scalar.dma_start(m0_sb[:], maskA[:])
        nc.scalar.dma_start(wvt[:], wd["v"][:].rearrange(
            "(cc p) j -> p cc j", p=128))
        for tb in range(1, 4):
            tsl = slice(tb * 512, (tb + 1) * 512)
            nc.scalar.dma_start(cos_sb[:, tsl], cosd[:, tsl])
            nc.scalar.dma_start(sin_sb[:, tsl], sind[:, tsl])
        nc.scalar.dma_start(wp_sb[:], wp[:].rearrange(
            "(cc p) co -> p cc co", p=128))

        ones_sb = const.tile([128, 64], bf16)
        nc.gpsimd.memset(ones_sb[:], 1.0)
        # warm the ACT exp table-set during input DMA
        warm_sb = const.tile([128, 16], f32)
        nc.vector.memset(warm_sb[:], 0.0)
        nc.scalar.activation(warm_sb[:], warm_sb[:], Exp)

        # rope'd Q^T/K^T halves; rows = 4 heads x 32 dims
        qke = {n: const.tile([128, T], bf16, tag=f"r_{n}", name=f"r_{n}")
               for n in ("qe", "qo", "ke", "ko")}
        v_sb = const.tile([128, 16, 256], bf16)   # V natural, t-chunks
        yab = const.tile([128, T], bf16)          # y^T heads A,B (normalized)
        ycd = const.tile([128, T], bf16)          # y^T heads C,D

        # pools (PSUM: sc 4 banks + y 2 + dn/rb 1 + blk 1 = 8)
        sc_ps = ctx.enter_context(tc.tile_pool(name="sc_ps", bufs=2, space="PSUM"))
        y_psp = ctx.enter_context(tc.tile_pool(name="y_ps", bufs=1, space="PSUM"))
        dn_ps = ctx.enter_context(tc.tile_pool(name="dn_ps", bufs=1, space="PSUM"))
        blk_ps = ctx.enter_context(tc.tile_pool(name="blk_ps", bufs=1, space="PSUM"))
        p_sbp = ctx.enter_context(tc.tile_pool(name="p_sb", bufs=4))
        rope_t = ctx.enter_context(tc.tile_pool(name="rope_t", bufs=3))
        r_sbp = ctx.enter_context(tc.tile_pool(name="r_sb", bufs=2))
        rbc_sbp = ctx.enter_context(tc.tile_pool(name="rbc_sb", bufs=3))
        pj_sbp = ctx.enter_context(tc.tile_pool(name="pj_sb", bufs=4))

        # ---------- emission helpers ----------
        def emit_qk(mk, tb):
            """QKV matmuls + rope for (q|k, t-block tb)."""
            tsl = slice(tb * 512, (tb + 1) * 512)
            ps = sc_ps.tile([128, 2, 512], f32, tag="scps")
            for cc in range(8):
                nc.tensor.matmul(ps[:, 0, :], lhsT=w_sb[mk + "e"][:, cc, :],
                                 rhs=xt_sb[:, cc, tsl],
                                 start=(cc == 0), stop=(cc == 7))
            for cc in range(8):
                nc.tensor.matmul(ps[:, 1, :], lhsT=w_sb[mk + "o"][:, cc, :],
                                 rhs=xt_sb[:, cc, tsl],
                                 start=(cc == 0), stop=(cc == 7))
            # evacuate PSUM via ACT (idle during QKV), then rope on SBUF
            # where DVE fp32 runs full rate.  E'=E*cos-O*sin ; O'=O*cos+E*sin
            cpe = rope_t.tile([128, 512], f32, tag="cpe")
            nc.vector.tensor_copy(cpe[:], ps[:, 0, :])
            cpo = rope_t.tile([128, 512], f32, tag="cpo")
            nc.vector.tensor_copy(cpo[:], ps[:, 1, :])
            a = rope_t.tile([128, 512], f32, tag="ra")
            nc.vector.tensor_tensor(a[:], cpe[:], cos_sb[:, tsl], AL.mult)
            b = rope_t.tile([128, 512], f32, tag="rb")
            nc.vector.tensor_tensor(b[:], cpo[:], sin_sb[:, tsl], AL.mult)
            nc.vector.tensor_tensor(qke[mk + "e"][:, tsl], a[:], b[:],
                                    AL.subtract)
            c = rope_t.tile([128, 512], f32, tag="rc")
            nc.vector.tensor_tensor(c[:], cpo[:], cos_sb[:, tsl], AL.mult)
            d = rope_t.tile([128, 512], f32, tag="rd")
            nc.vector.tensor_tensor(d[:], cpe[:], sin_sb[:, tsl], AL.mult)
            nc.vector.tensor_tensor(qke[mk + "o"][:, tsl], c[:], d[:], AL.add)

        def emit_vpair(vp):
            """V matmuls for t-chunks 2*vp, 2*vp+1 through one blk rotation."""
            bt = blk_ps.tile([128, 512], f32, tag="blk")
            for k in range(2):
                tcx = 2 * vp + k
                tchunk = slice(tcx * 128, (tcx + 1) * 128)
                for cc in range(8):
                    nc.tensor.matmul(bt[:, 256 * k:256 * (k + 1)],
                                     lhsT=xt_sb[:, cc, tchunk],
                                     rhs=w_sb["v"][:, cc, :],
                                     start=(cc == 0), stop=(cc == 7),
                                     skip_group_check=True)
                nc.vector.tensor_copy(v_sb[:, tcx, :],
                                      bt[:, 256 * k:256 * (k + 1)])

        qe_sb, qo_sb = qke["qe"], qke["qo"]
        ke_sb, ko_sb = qke["ke"], qke["ko"]

        def emit_scores(ib, jc):
            """scores + exp for (ib, jc); returns the two pt pair-tiles."""
            njc = 4 * ib + 4
            jsl = slice(jc * JCS, (jc + 1) * JCS)
            jcd = jc - 4 * ib
            off = 128 * jcd if jcd > 0 else 0
            osl = slice(off, 512)
            iosl = slice(ib * IBS + off, (ib + 1) * IBS)
            pts = []
            for half in range(2):
                sch = sc_ps.tile([128, 2, 512], f32, tag="scps")
                for hh in range(2):
                    h = 2 * half + hh
                    hsl = slice(32 * h, 32 * h + 32)
                    nc.tensor.matmul(
                        sch[:, hh, osl], lhsT=ke_sb[hsl, jsl],
                        rhs=qe_sb[hsl, iosl], start=True, stop=False,
                        tile_position=(32 * h, 0))
                for hh in range(2):
                    h = 2 * half + hh
                    hsl = slice(32 * h, 32 * h + 32)
                    nc.tensor.matmul(
                        sch[:, hh, osl], lhsT=ko_sb[hsl, jsl],
                        rhs=qo_sb[hsl, iosl], start=False, stop=True,
                        tile_position=(32 * h, 0))
                pt = p_sbp.tile([128, 2, 512], bf16, tag=f"pt{half}")
                nc.scalar.activation(pt[:, :, osl], sch[:, :, osl], Exp,
                                     scale=scale)
                if jcd >= 0:
                    tsl = slice(off, off + 128)
                    nc.vector.tensor_tensor(
                        pt[:, :, tsl], pt[:, :, tsl],
                        m0_sb[:, None, :].broadcast_to([128, 2, 128]),
                        AL.mult)
                pts.append(pt)
            return pts

        def emit_avdn(ib, jc, pts, y_ps, dn):
            njc = 4 * ib + 4
            jcd = jc - 4 * ib
            off = 128 * jcd if jcd > 0 else 0
            osl = slice(off, 512)
            for pair in range(2):
                for k in range(2):
                    h = 2 * pair + k
                    nc.tensor.matmul(
                        y_ps[64 * k:64 * (k + 1), pair, osl],
                        lhsT=v_sb[:, jc, 64 * h:64 * (h + 1)],
                        rhs=pts[pair][:, k, osl],
                        start=(jc == 0), stop=(jc == njc - 1),
                        tile_position=(0, 64 * k),
                        skip_group_check=True)
            for h in range(4):
                nc.tensor.matmul(
                    dn[32 * h:32 * h + 1, osl],
                    lhsT=ones_sb[:, 0:1],
                    rhs=pts[h // 2][:, h % 2, osl],
                    start=(jc == 0), stop=(jc == njc - 1),
                    tile_position=(0, 32 * h),
                    skip_group_check=True)

        def emit_tail1(ib, dn):
            """1/dn as exp(-ln(dn)), both on ACT: Ln reads PSUM directly
            (fast bank release), Exp writes bf16 -- ~1.5us chain with no
            DVE involvement, vs copy+iterative-reciprocal (~4.4us)."""
            t = r_sbp.tile([128, 512], f32, tag="dnsb")
            nc.scalar.activation(t[:], dn[:], Ln)
            r16 = r_sbp.tile([128, 512], bf16, tag="r16")
            nc.scalar.activation(r16[:], t[:], Exp, scale=-1.0)
            return r16

        def emit_tail2(ib, y_ps, r16):
            """rbcast + normalize; deferred into the next ib's fill stream
            so the PE never queues behind the reciprocal."""
            isl = slice(ib * IBS, (ib + 1) * IBS)
            for pair, ytile in ((0, yab), (1, ycd)):
                rb = blk_ps.tile([128, 512], f32, tag="blk")
                for k in range(2):
                    h = 2 * pair + k
                    nc.tensor.matmul(
                        rb[64 * k:64 * (k + 1), :],
                        lhsT=ones_sb[32 * h:32 * h + 1, :],
                        rhs=r16[32 * h:32 * h + 1, :],
                        start=True, stop=True,
                        tile_position=(32 * h, 64 * k),
                        skip_group_check=True)
                rbc = rbc_sbp.tile([128, 512], f32, tag="rbc")
                nc.vector.tensor_copy(rbc[:], rb[:])
                nc.vector.tensor_tensor(ytile[:, isl], y_ps[:, pair, :],
                                        rbc[:], AL.mult)

        def emit_proj_block(tcx, cob, fast=False):
            tchunk = slice(tcx * 128, (tcx + 1) * 128)
            cosl = slice(cob * 512, (cob + 1) * 512)
            if fast:  # epilogue: sc pool is free, use it double-buffered
                ppt = sc_ps.tile([128, 2, 512], f32, tag="scps", name="pjf")
                pp = ppt[:, 0, :]
            else:
                pp = blk_ps.tile([128, 512], f32, tag="blk")
            nc.tensor.matmul(pp[:], lhsT=yab[:, tchunk],
                             rhs=wp_sb[:, 0, cosl], start=True, stop=False)
            nc.tensor.matmul(pp[:], lhsT=ycd[:, tchunk],
                             rhs=wp_sb[:, 1, cosl], start=False, stop=True)
            ps = pj_sbp.tile([128, 512], bf16, tag="pjsb")
            nc.vector.tensor_copy(ps[:], pp[:])
            nc.sync.dma_start(out[tchunk, cosl], ps[:])

        # ---------- main emission schedule ----------
        # Fill queue: QKV/V/proj work drained one unit per jc iteration,
        # emitted between scores(jc) and avdn(jc-1) so it lands exactly in
        # the PE's exp-wait window instead of queuing behind it.
        fillq = []
        tail2_pending = []   # at most one deferred (ib, y_ps, r16)

        def flush_tail2():
            if tail2_pending:
                emit_tail2(*tail2_pending.pop())

        def emit_attention(ib):
            njc = 4 * ib + 4
            acc = {}

            def ensure_acc():
                if "y" not in acc:
                    # previous ib's rbcast/normalize must be emitted before
                    # this ib claims the single y_ps buffer
                    flush_tail2()
                    acc["y"] = y_psp.tile([128, 2, 512], f32, tag="yps",
                                          name="yps")
                    acc["dn"] = dn_ps.tile([128, 512], f32, tag="dnrb",
                                           name="dnrb")

            pending = None
            for jc in range(njc):
                pts = emit_scores(ib, jc)
                if fillq:
                    fillq.pop(0)()
                if pending is not None:
                    ensure_acc()
                    emit_avdn(ib, pending[0], pending[1], acc["y"], acc["dn"])
                pending = (jc, pts)
            ensure_acc()
            emit_avdn(ib, pending[0], pending[1], acc["y"], acc["dn"])
            r16 = emit_tail1(ib, acc["dn"])
            tail2_pending.append((ib, acc["y"], r16))
            for tcx in range(4 * ib, 4 * ib + 4):
                for cob in range(2):
                    fillq.append(
                        lambda fast=False, t=tcx, c=cob:
                        emit_proj_block(t, c, fast))

        # eager prologue: QKV for t-block 0 only; all the rest of QKV/V
        # rides the fill queue so attention(0)'s exp pipeline starts ASAP
        for mk in ("k", "q"):
            emit_qk(mk, 0)
        fillq.extend([
            lambda: emit_vpair(0), lambda: emit_vpair(1),
            lambda: emit_qk("k", 1), lambda: emit_qk("q", 1),
        ])
        emit_attention(0)
        fillq[:0] = [
            lambda: emit_qk("k", 2), lambda: emit_qk("q", 2),
            lambda: emit_vpair(2), lambda: emit_vpair(3),
        ]
        emit_attention(1)
        fillq[:0] = [
            lambda: emit_qk("k", 3), lambda: emit_qk("q", 3),
            lambda: emit_vpair(4), lambda: emit_vpair(5),
        ]
        emit_attention(2)
        fillq[:0] = [
            lambda: emit_vpair(6), lambda: emit_vpair(7),
        ]
        emit_attention(3)
        flush_tail2()
        while fillq:  # epilogue: proj(ib3) blocks, pipelined through sc pool
            fillq.pop(0)(fast=True)
    return nc


def _host_prep(x, cos, sin, W_attn, W_proj):
    """Build the 8 per-core input maps (pure data movement / layout prep)."""
    import ml_dtypes
    bf16 = ml_dtypes.bfloat16
    x = np.asarray(x)
    cos = np.asarray(cos)
    sin = np.asarray(sin)
    W_attn = np.asarray(W_attn)
    W_proj = np.asarray(W_proj)

    cosf = np.ascontiguousarray(cos[0, 0][:, 0::2].T.astype(np.float32))  # [32,T]
    sinf = np.ascontiguousarray(sin[0, 0][:, 0::2].T.astype(np.float32))
    cosd = np.tile(cosf, (4, 1)).astype(bf16)  # [128, T]
    sind = np.tile(sinf, (4, 1)).astype(bf16)

    mA = (np.arange(128)[:, None] <= np.arange(128)[None, :]).astype(np.float32)

    ev = np.arange(0, HD, 2)
    od = np.arange(1, HD, 2)
    Wq, Wk, Wv = W_attn[:, 0:C], W_attn[:, C:2 * C], W_attn[:, 2 * C:3 * C]
    xt = [np.ascontiguousarray(x[b].T).astype(bf16) for b in range(B)]

    in_maps = []
    for g in range(NCORES):
        b, hg = g // HPC, g % HPC
        heads = [HPC * hg + i for i in range(HPC)]
        mk = lambda W, idx: np.ascontiguousarray(
            np.concatenate([W[:, 64 * h + idx] for h in heads], 1)).astype(bf16)
        in_maps.append({
            "xt": xt[b],
            "wqe": mk(Wq, ev), "wqo": mk(Wq, od),
            "wke": mk(Wk, ev), "wko": mk(Wk, od),
            "wv": mk(Wv, np.arange(HD)),
            "cosd": cosd, "sind": sind, "maskA": mA.astype(bf16),
            "wp": np.ascontiguousarray(np.concatenate(
                [W_proj[64 * h:64 * h + 64, :] for h in heads], 0)
            ).astype(bf16),
        })
    return in_maps


def _run(inputs, trace=False):
    from concourse.bass_utils import run_bass_kernel_spmd

    if "nc" not in _cache:
        _cache["nc"] = build_nc()
    nc = _cache["nc"]
    in_maps = _host_prep(**inputs)
    res = run_bass_kernel_spmd(
        nc, in_maps, core_ids=list(range(NCORES)), trace=trace)
    outp = np.stack([np.asarray(res.results[g]["out"], dtype=np.float32)
                     for g in range(NCORES)])
    full = np.stack([outp[4 * b:4 * b + 4].sum(axis=0) for b in range(B)])
    return full, res


def kernel(**inputs):
    full, _ = _run(inputs, trace=False)
    return full


# revision 62
# speedup vs baseline: 1.0135x; 1.0028x over previous
"""Causal self-attention (B=2, T=2048, C=1024, NH=16, HD=64) on 8 trn2 NeuronCores.

Sharding: 2 batch groups x 4 head-groups. Core g (0..7) handles batch b=g//4
and heads [4*(g%4), 4*(g%4)+4). Each core computes its 4 heads' attention and a
partial projection (row-split W_proj); the host sums 4 partials per batch.

v2: software-pipelined emission so the PE never queues behind EXP latency:
  - x DMA'd in t-blocks; QKV for t-block tb interleaves with attention i-block
    tb-1, so ACT's exp pipeline starts ~11us into the kernel.
  - scores(jc+1) emitted before AV/dn(jc): EXP latency hides under the next
    chunk's scores matmuls; PE stays dense (HAM stays warm at 2.4GHz).
  - pt split per head-pair so AV pair p waits only its own exp.
  - reciprocal_approx_fast, rbc evacuation on ACT, bf16 output (half DMA),
    projection for i-block ib interleaved into i-block ib+1's jc loop.
"""

import numpy as np

B, T, C, NH, HD = 2, 2048, 1024, 16, 64
NCORES = 8
HPC = 4          # heads per core
IBS = 512        # i-block (query block) size
NIB = T // IBS   # 4 i-blocks
JCS = 128        # j-chunk (key chunk) size

_cache = {}


def _patch_tile_drain():
    """This walrus build can't encode multi-wait InstDrain: split the Tile
    tail drain into a chain of single-wait drains."""
    import concourse.tile as _tile
    if getattr(_tile.TileContext, "_drain_patched", False):
        return
    import bass_rust as _br
    from concourse.vector_clock import ScopedClock

    def _drain_and_barrier(self, tick_clock, wait_clock):
        nc = self.nc
        drain_inst = nc.sync.drain()
        wait_clock.add_sem_waits(
            drain_inst.ins, ScopedClock({None: tick_clock.global_clock})
        )
        si = drain_inst.ins.sync_info
        waits = list(si.on_wait or [])
        if len(waits) > 1:
            si.on_wait = waits[:1]
            for w in waits[1:]:
                extra = nc.sync.drain()
                extra.ins.sync_info = _br.SyncInfo(on_wait=[w], on_update=[])
        nc.all_engine_barrier()
        assert self.sems is not None
        popped = nc._tile_sem_poison_stack.pop()
        assert popped is self._sem_poison
        nc.clear_and_free_semaphores(list(self.sems.allocated().values()))
        nc.all_engine_barrier()

    _tile.TileContext._drain_and_barrier = _drain_and_barrier

    # This walrus also refuses >1 sem wait on ANY instruction: peel extra
    # waits onto ENGINE_NOP carriers inserted just before, same engine/bb.
    _orig_add = _tile.TileContext._add_instruction

    def _add_instruction(self, inst):
        si = getattr(inst, "sync_info", None)
        if si is not None and si.on_wait and len(si.on_wait) > 1:
            waits = list(si.on_wait)
            si.on_wait = waits[-1:]
            import concourse.mybir as _mb
            for w in waits[:-1]:
                nop = _mb.InstEventSemaphore(
                    name=self.nc.get_next_instruction_name(), ins=[], outs=[])
                nop.engine = inst.engine
                nop.sync_info = _br.SyncInfo(on_wait=[w], on_update=[])
                _orig_add(self, nop)
        _orig_add(self, inst)

    _tile.TileContext._add_instruction = _add_instruction
    _tile.TileContext._drain_patched = True


def build_nc():
    import concourse.bass as bass
    import concourse.mybir as mybir
    import concourse.tile as tile
    from contextlib import ExitStack

    _patch_tile_drain()
    dt = mybir.dt
    f32, bf16 = dt.float32, dt.bfloat16
    AL = mybir.AluOpType
    Exp = mybir.ActivationFunctionType.Exp
    Ln = mybir.ActivationFunctionType.Ln
    nc = bass.Bass()

    xt = nc.declare_dram_parameter("xt", [C, T], bf16, isOutput=False)
    wd = {}
    for name, w in (("qe", 128), ("qo", 128), ("ke", 128), ("ko", 128),
                    ("v", 256)):
        wd[name] = nc.declare_dram_parameter(f"w{name}", [C, w], bf16,
                                             isOutput=False)
    cosd = nc.declare_dram_parameter("cosd", [128, T], bf16, isOutput=False)
    sind = nc.declare_dram_parameter("sind", [128, T], bf16, isOutput=False)
    maskA = nc.declare_dram_parameter("maskA", [128, 128], bf16, isOutput=False)
    wp = nc.declare_dram_parameter("wp", [256, C], bf16, isOutput=False)
    out = nc.declare_dram_parameter("out", [T, C], bf16, isOutput=True)

    scale = 1.0 / float(np.sqrt(HD))

    with tile.TileContext(nc) as tc, ExitStack() as ctx:
        const = ctx.enter_context(tc.tile_pool(name="const", bufs=1))

        # ---------- DMA: two HWDGE rings (sync + scalar), big transfers,
        # earliest-needed first ----------
        w_sb = {}
        for name in ("ke", "ko", "qe", "qo"):
            w_sb[name] = const.tile([128, 8, 128], bf16, tag=f"w_{name}",
                                    name=f"w_{name}")
        cos_sb = const.tile([128, T], bf16)
        sin_sb = const.tile([128, T], bf16)
        xt_sb = const.tile([128, 8, T], bf16)
        xt_r = xt[:].rearrange("(cc p) t -> p cc t", p=128)
        m0_sb = const.tile([128, 128], bf16)
        wvt = const.tile([128, 8, 256], bf16, tag="w_v", name="w_v")
        w_sb["v"] = wvt
        wp_sb = const.tile([128, 2, C], bf16)
        tsl0 = slice(0, 512)
        # sync ring: wke, then x tb0 first half (unblocks the first e-MMs),
        # then wko, then the rest
        nc.sync.dma_start(w_sb["ke"][:], wd["ke"][:].rearrange(
            "(cc p) j -> p cc j", p=128))
        nc.sync.dma_start(xt_sb[:, 0:4, tsl0], xt_r[:, 0:4, tsl0])
        nc.sync.dma_start(w_sb["ko"][:], wd["ko"][:].rearrange(
            "(cc p) j -> p cc j", p=128))
        nc.sync.dma_start(xt_sb[:, 4:8, tsl0], xt_r[:, 4:8, tsl0])
        for tb in range(1, 4):
            tsl = slice(tb * 512, (tb + 1) * 512)
            nc.sync.dma_start(xt_sb[:, :, tsl], xt_r[:, :, tsl])
        # scalar ring: q weights, rope tables, mask, v/proj weights
        for name in ("qe", "qo"):
            nc.scalar.dma_start(w_sb[name][:], wd[name][:].rearrange(
                "(cc p) j -> p cc j", p=128))
        nc.scalar.dma_start(cos_sb[:, tsl0], cosd[:, tsl0])
        nc.scalar.dma_start(sin_sb[:, tsl0], sind[:, tsl0])
        nc.scalar.dma_start(m0_sb[:], maskA[:])
        nc.scalar.dma_start(wvt[:], wd["v"][:].rearrange(
            "(cc p) j -> p cc j", p=128))
        for tb in range(1, 4):
            tsl = slice(tb * 512, (tb + 1) * 512)
            nc.scalar.dma_start(cos_sb[:, tsl], cosd[:, tsl])
            nc.scalar.dma_start(sin_sb[:, tsl], sind[:, tsl])
        nc.scalar.dma_start(wp_sb[:], wp[:].rearrange(
            "(cc p) co -> p cc co", p=128))

        ones_sb = const.tile([128, 64], bf16)
        nc.gpsimd.memset(ones_sb[:], 1.0)
        # warm the ACT exp table-set during input DMA
        warm_sb = const.tile([128, 16], f32)
        nc.vector.memset(warm_sb[:], 0.0)
        nc.scalar.activation(warm_sb[:], warm_sb[:], Exp)

        # rope'd Q^T/K^T halves; rows = 4 heads x 32 dims
        qke = {n: const.tile([128, T], bf16, tag=f"r_{n}", name=f"r_{n}")
               for n in ("qe", "qo", "ke", "ko")}
        v_sb = const.tile([128, 16, 256], bf16)   # V natural, t-chunks
        yab = const.tile([128, T], bf16)          # y^T heads A,B (normalized)
        ycd = const.tile([128, T], bf16)          # y^T heads C,D

        # pools (PSUM: sc 4 banks + y 2 + dn/rb 1 + blk 1 = 8)
        sc_ps = ctx.enter_context(tc.tile_pool(name="sc_ps", bufs=2, space="PSUM"))
        y_psp = ctx.enter_context(tc.tile_pool(name="y_ps", bufs=1, space="PSUM"))
        dn_ps = ctx.enter_context(tc.tile_pool(name="dn_ps", bufs=1, space="PSUM"))
        blk_ps = ctx.enter_context(tc.tile_pool(name="blk_ps", bufs=1, space="PSUM"))
        p_sbp = ctx.enter_context(tc.tile_pool(name="p_sb", bufs=4))
        rope_t = ctx.enter_context(tc.tile_pool(name="rope_t", bufs=3))
        r_sbp = ctx.enter_context(tc.tile_pool(name="r_sb", bufs=2))
        rbc_sbp = ctx.enter_context(tc.tile_pool(name="rbc_sb", bufs=3))
        pj_sbp = ctx.enter_context(tc.tile_pool(name="pj_sb", bufs=4))

        # ---------- emission helpers ----------
        def emit_qk(mk, tb):
            """QKV matmuls + rope for (q|k, t-block tb)."""
            tsl = slice(tb * 512, (tb + 1) * 512)
            ps = sc_ps.tile([128, 2, 512], f32, tag="scps")
            for cc in range(8):
                nc.tensor.matmul(ps[:, 0, :], lhsT=w_sb[mk + "e"][:, cc, :],
                                 rhs=xt_sb[:, cc, tsl],
                                 start=(cc == 0), stop=(cc == 7))
                nc.tensor.matmul(ps[:, 1, :], lhsT=w_sb[mk + "o"][:, cc, :],
                                 rhs=xt_sb[:, cc, tsl],
                                 start=(cc == 0), stop=(cc == 7))
            # evacuate PSUM via ACT (idle during QKV), then rope on SBUF
            # where DVE fp32 runs full rate.  E'=E*cos-O*sin ; O'=O*cos+E*sin
            cpe = rope_t.tile([128, 512], f32, tag="cpe")
            nc.vector.tensor_copy(cpe[:], ps[:, 0, :])
            cpo = rope_t.tile([128, 512], f32, tag="cpo")
            nc.vector.tensor_copy(cpo[:], ps[:, 1, :])
            a = rope_t.tile([128, 512], f32, tag="ra")
            nc.vector.tensor_tensor(a[:], cpe[:], cos_sb[:, tsl], AL.mult)
            b = rope_t.tile([128, 512], f32, tag="rb")
            nc.vector.tensor_tensor(b[:], cpo[:], sin_sb[:, tsl], AL.mult)
            nc.vector.tensor_tensor(qke[mk + "e"][:, tsl], a[:], b[:],
                                    AL.subtract)
            c = rope_t.tile([128, 512], f32, tag="rc")
            nc.vector.tensor_tensor(c[:], cpo[:], cos_sb[:, tsl], AL.mult)
            d = rope_t.tile([128, 512], f32, tag="rd")
            nc.vector.tensor_tensor(d[:], cpe[:], sin_sb[:, tsl], AL.mult)
            nc.vector.tensor_tensor(qke[mk + "o"][:, tsl], c[:], d[:], AL.add)

        def emit_vpair(vp):
            """V matmuls for t-chunks 2*vp, 2*vp+1 through one blk rotation."""
            bt = blk_ps.tile([128, 512], f32, tag="blk")
            for k in range(2):
                tcx = 2 * vp + k
                tchunk = slice(tcx * 128, (tcx + 1) * 128)
                for cc in range(8):
                    nc.tensor.matmul(bt[:, 256 * k:256 * (k + 1)],
                                     lhsT=xt_sb[:, cc, tchunk],
                                     rhs=w_sb["v"][:, cc, :],
                                     start=(cc == 0), stop=(cc == 7),
                                     skip_group_check=True)
                nc.vector.tensor_copy(v_sb[:, tcx, :],
                                      bt[:, 256 * k:256 * (k + 1)])

        qe_sb, qo_sb = qke["qe"], qke["qo"]
        ke_sb, ko_sb = qke["ke"], qke["ko"]

        def emit_scores(ib, jc):
            """scores + exp for (ib, jc); returns the two pt pair-tiles."""
            njc = 4 * ib + 4
            jsl = slice(jc * JCS, (jc + 1) * JCS)
            jcd = jc - 4 * ib
            off = 128 * jcd if jcd > 0 else 0
            osl = slice(off, 512)
            iosl = slice(ib * IBS + off, (ib + 1) * IBS)
            pts = []
            for half in range(2):
                sch = sc_ps.tile([128, 2, 512], f32, tag="scps")
                for hh in range(2):
                    h = 2 * half + hh
                    hsl = slice(32 * h, 32 * h + 32)
                    nc.tensor.matmul(
                        sch[:, hh, osl], lhsT=ke_sb[hsl, jsl],
                        rhs=qe_sb[hsl, iosl], start=True, stop=False,
                        tile_position=(32 * h, 0))
                for hh in range(2):
                    h = 2 * half + hh
                    hsl = slice(32 * h, 32 * h + 32)
                    nc.tensor.matmul(
                        sch[:, hh, osl], lhsT=ko_sb[hsl, jsl],
                        rhs=qo_sb[hsl, iosl], start=False, stop=True,
                        tile_position=(32 * h, 0))
                pt = p_sbp.tile([128, 2, 512], bf16, tag=f"pt{half}")
                nc.scalar.activation(pt[:, :, osl], sch[:, :, osl], Exp,
                                     scale=scale)
                if jcd >= 0:
                    tsl = slice(off, off + 128)
                    nc.vector.tensor_tensor(
                        pt[:, :, tsl], pt[:, :, tsl],
                        m0_sb[:, None, :].broadcast_to([128, 2, 128]),
                        AL.mult)
                pts.append(pt)
            return pts

        def emit_avdn(ib, jc, pts, y_ps, dn):
            njc = 4 * ib + 4
            jcd = jc - 4 * ib
            off = 128 * jcd if jcd > 0 else 0
            osl = slice(off, 512)
            for pair in range(2):
                for k in range(2):
                    h = 2 * pair + k
                    nc.tensor.matmul(
                        y_ps[64 * k:64 * (k + 1), pair, osl],
                        lhsT=v_sb[:, jc, 64 * h:64 * (h + 1)],
                        rhs=pts[pair][:, k, osl],
                        start=(jc == 0), stop=(jc == njc - 1),
                        tile_position=(0, 64 * k),
                        skip_group_check=True)
            for h in range(4):
                nc.tensor.matmul(
                    dn[32 * h:32 * h + 1, osl],
                    lhsT=ones_sb[:, 0:1],
                    rhs=pts[h // 2][:, h % 2, osl],
                    start=(jc == 0), stop=(jc == njc - 1),
                    tile_position=(0, 32 * h),
                    skip_group_check=True)

        def emit_tail1(ib, dn):
            """1/dn as exp(-ln(dn)), both on ACT: Ln reads PSUM directly
            (fast bank release), Exp writes bf16 -- ~1.5us chain with no
            DVE involvement, vs copy+iterative-reciprocal (~4.4us)."""
            t = r_sbp.tile([128, 512], f32, tag="dnsb")
            nc.scalar.activation(t[:], dn[:], Ln)
            r16 = r_sbp.tile([128, 512], bf16, tag="r16")
            nc.scalar.activation(r16[:], t[:], Exp, scale=-1.0)
            return r16

        def emit_tail2(ib, y_ps, r16):
            """rbcast + normalize; deferred into the next ib's fill stream
            so the PE never queues behind the reciprocal."""
            isl = slice(ib * IBS, (ib + 1) * IBS)
            for pair, ytile in ((0, yab), (1, ycd)):
                rb = blk_ps.tile([128, 512], f32, tag="blk")
                for k in range(2):
                    h = 2 * pair + k
                    nc.tensor.matmul(
                        rb[64 * k:64 * (k + 1), :],
                        lhsT=ones_sb[32 * h:32 * h + 1, :],
                        rhs=r16[32 * h:32 * h + 1, :],
                        start=True, stop=True,
                        tile_position=(32 * h, 64 * k),
                        skip_group_check=True)
                rbc = rbc_sbp.tile([128, 512], f32, tag="rbc")
                nc.vector.tensor_copy(rbc[:], rb[:])
                nc.vector.tensor_tensor(ytile[:, isl], y_ps[:, pair, :],
                                        rbc[:], AL.mult)

        def emit_proj_block(tcx, cob, fast=False):
            tchunk = slice(tcx * 128, (tcx + 1) * 128)
            cosl = slice(cob * 512, (cob + 1) * 512)
            if fast:  # epilogue: sc pool is free, use it double-buffered
                ppt = sc_ps.tile([128, 2, 512], f32, tag="scps", name="pjf")
                pp = ppt[:, 0, :]
            else:
                pp = blk_ps.tile([128, 512], f32, tag="blk")
            nc.tensor.matmul(pp[:], lhsT=yab[:, tchunk],
                             rhs=wp_sb[:, 0, cosl], start=True, stop=False)
            nc.tensor.matmul(pp[:], lhsT=ycd[:, tchunk],
                             rhs=wp_sb[:, 1, cosl], start=False, stop=True)
            ps = pj_sbp.tile([128, 512], bf16, tag="pjsb")
            nc.vector.tensor_copy(ps[:], pp[:])
            nc.sync.dma_start(out[tchunk, cosl], ps[:])

        # ---------- main emission schedule ----------
        # Fill queue: QKV/V/proj work drained one unit per jc iteration,
        # emitted between scores(jc) and avdn(jc-1) so it lands exactly in
        # the PE's exp-wait window instead of queuing behind it.
        fillq = []
        tail2_pending = []   # at most one deferred (ib, y_ps, r16)

        def flush_tail2():
            if tail2_pending:
                emit_tail2(*tail2_pending.pop())

        def emit_attention(ib):
            njc = 4 * ib + 4
            acc = {}

            def ensure_acc():
                if "y" not in acc:
                    # previous ib's rbcast/normalize must be emitted before
                    # this ib claims the single y_ps buffer
                    flush_tail2()
                    acc["y"] = y_psp.tile([128, 2, 512], f32, tag="yps",
                                          name="yps")
                    acc["dn"] = dn_ps.tile([128, 512], f32, tag="dnrb",
                                           name="dnrb")

            pending = None
            for jc in range(njc):
                pts = emit_scores(ib, jc)
                if fillq:
                    fillq.pop(0)()
                if pending is not None:
                    ensure_acc()
                    emit_avdn(ib, pending[0], pending[1], acc["y"], acc["dn"])
                pending = (jc, pts)
            ensure_acc()
            emit_avdn(ib, pending[0], pending[1], acc["y"], acc["dn"])
            r16 = emit_tail1(ib, acc["dn"])
            tail2_pending.append((ib, acc["y"], r16))
            for tcx in range(4 * ib, 4 * ib + 4):
                for cob in range(2):
                    fillq.append(
                        lambda fast=False, t=tcx, c=cob:
                        emit_proj_block(t, c, fast))

        # eager prologue: QKV for t-block 0 only; all the rest of QKV/V
        # rides the fill queue so attention(0)'s exp pipeline starts ASAP
        for mk in ("k", "q"):
            emit_qk(mk, 0)
        fillq.extend([
            lambda: emit_qk("k", 1), lambda: emit_vpair(0),
            lambda: emit_qk("q", 1), lambda: emit_vpair(1),
        ])
        emit_attention(0)
        fillq[:0] = [
            lambda: emit_qk("k", 2), lambda: emit_qk("q", 2),
            lambda: emit_vpair(2), lambda: emit_vpair(3),
        ]
        emit_attention(1)
        fillq[:0] = [
            lambda: emit_qk("k", 3), lambda: emit_qk("q", 3),
            lambda: emit_vpair(4), lambda: emit_vpair(5),
        ]
        emit_attention(2)
        fillq[:0] = [
            lambda: emit_vpair(6), lambda: emit_vpair(7),
        ]
        emit_attention(3)
        flush_tail2()
        while fillq:  # epilogue: proj(ib3) blocks, pipelined through sc pool
            fillq.pop(0)(fast=True)
    return nc


def _host_prep(x, cos, sin, W_attn, W_proj):
    """Build the 8 per-core input maps (pure data movement / layout prep)."""
    import ml_dtypes
    bf16 = ml_dtypes.bfloat16
    x = np.asarray(x)
    cos = np.asarray(cos)
    sin = np.asarray(sin)
    W_attn = np.asarray(W_attn)
    W_proj = np.asarray(W_proj)

    cosf = np.ascontiguousarray(cos[0, 0][:, 0::2].T.astype(np.float32))  # [32,T]
    sinf = np.ascontiguousarray(sin[0, 0][:, 0::2].T.astype(np.float32))
    cosd = np.tile(cosf, (4, 1)).astype(bf16)  # [128, T]
    sind = np.tile(sinf, (4, 1)).astype(bf16)

    mA = (np.arange(128)[:, None] <= np.arange(128)[None, :]).astype(np.float32)

    ev = np.arange(0, HD, 2)
    od = np.arange(1, HD, 2)
    Wq, Wk, Wv = W_attn[:, 0:C], W_attn[:, C:2 * C], W_attn[:, 2 * C:3 * C]
    xt = [np.ascontiguousarray(x[b].T).astype(bf16) for b in range(B)]

    in_maps = []
    for g in range(NCORES):
        b, hg = g // HPC, g % HPC
        heads = [HPC * hg + i for i in range(HPC)]
        mk = lambda W, idx: np.ascontiguousarray(
            np.concatenate([W[:, 64 * h + idx] for h in heads], 1)).astype(bf16)
        in_maps.append({
            "xt": xt[b],
            "wqe": mk(Wq, ev), "wqo": mk(Wq, od),
            "wke": mk(Wk, ev), "wko": mk(Wk, od),
            "wv": mk(Wv, np.arange(HD)),
            "cosd": cosd, "sind": sind, "maskA": mA.astype(bf16),
            "wp": np.ascontiguousarray(np.concatenate(
                [W_proj[64 * h:64 * h + 64, :] for h in heads], 0)
            ).astype(bf16),
        })
    return in_maps


def _run(inputs, trace=False):
    from concourse.bass_utils import run_bass_kernel_spmd

    if "nc" not in _cache:
        _cache["nc"] = build_nc()
    nc = _cache["nc"]
    in_maps = _host_prep(**inputs)
    res = run_bass_kernel_spmd(
        nc, in_maps, core_ids=list(range(NCORES)), trace=trace)
    outp = np.stack([np.asarray(res.results[g]["out"], dtype=np.float32)
                     for g in range(NCORES)])
    full = np.stack([outp[4 * b:4 * b + 4].sum(axis=0) for b in range(B)])
    return full, res


def kernel(**inputs):
    full, _ = _run(inputs, trace=False)
    return full
